# revision 14
# baseline (speedup 1.0000x reference)
"""Trainium2 Bass kernel for nn_CustomANFIS (N=4096, D=128, R=256, O=64).

Math (reference):
  memb[n,r,d]  = exp(-(x[n,d]-c[r,d])^2 / (2 s[r,d]^2))
  str[n,r]     = prod_d memb = exp(z[n,r]) with
                 z[n,r] = -(sum_d x^2 A + sum_d x B + G),
                 A = 1/(2 s^2), B = -c/s^2, G = sum_d c^2/(2 s^2)
  den[n]       = sum_r str + 1e-8
  out          = softmax_j( (1/den) * sum_r str[n,r] * (X@C_r + b_r) )

Device algorithm (data-parallel over N across 8 cores, NS=512 rows/core):
  1. z[n,r] via 3 accumulating fp32r matmuls per n-tile ([n,256] PSUM,
     G folded in via a K=1 ones-row matmul); zmax = max_r z (DVE);
     st2[n,r] = exp(z - zmax) written as fp8e4 (ACT, per-partition bias).
     Per-n renorm keeps st in [0,1] for fp8; exactness restored via
     eps'[n] = 64e-8 * exp(-zmax) added to the 64-scaled den.
  2. st2 -> st3[r-in-ktile, kt, n] via PE transposes (fp8), ACT evac.
  3. den+bias consequent: one DoubleRow fp8 matmul per n-tile
     (lhsT=st3 slice, rhs=cbo with 64-scaled ones/bias cols).
  4. Big contraction TRANSPOSED (d on partitions): per j, one DoubleRow
     fp8 matmul T2_j[d, n] = sum_r C64[r,d,j] st[r,n], j-pairs (j, j+32)
     share a [128, 2, 512] PSUM tile.
  5. X-multiply: ACT evac pair -> bf16 (or DVE direct from PSUM), then
     DVE/GpSimd multiply by Xt[d,n] (broadcast over the pair slot).
  6. d-reduction ON THE PE: ones-column matmuls (lhsT = eye32 slice
     replicated over partitions) accumulate row m of res[32, 2, 512]
     = out^T. No DVE tree.
  7. res -> PE transposes -> logits[n, j] (bf16 PSUM); + bias (tb),
     softmax with scale folded (1/(64 den)), DMA out.
"""

import numpy as np
import ml_dtypes

N, D, R, O = 4096, 128, 256, 64
NCORES = 8
NS = N // NCORES          # 512 rows per core
NT = NS // 128            # 4 n-tiles per core
RT = R // 128             # 2 r k-tiles
DJ = D * O                # 8192
NUNIT = 32                # j-pair units: unit m covers j=m and j=m+32
CSCALE = 64.0
LN_EPS64 = float(np.log(CSCALE * 1e-8))   # ln(64e-8) = -14.2618...

# route per unit index 0..31: 'a' = ACT evac + DVE mult,
# 'b' = DVE mult direct from PSUM, 'd' = ACT evac + GPS mult
ROUTES = (
    "a b a d a b a d a b a d a b a d a b a d a b a d a b d a b a d a".split()
)

_CACHE = {}
BF16 = ml_dtypes.bfloat16
FP8 = ml_dtypes.float8_e4m3


def _build():
    import concourse.bass as bass
    import concourse.tile as tile
    from concourse import bacc, mybir

    f32 = mybir.dt.float32
    f32r = mybir.dt.float32r
    bf16 = mybir.dt.bfloat16
    fp8 = mybir.dt.float8e4
    AF = mybir.ActivationFunctionType
    ALU = mybir.AluOpType
    DR = mybir.MatmulPerfMode.DoubleRow
    ts = bass.ts

    nc = bacc.Bacc(
        "TRN2", target_bir_lowering=False, debug=False, num_devices=NCORES
    )

    xt_d = nc.dram_tensor("xt", [D, NS], f32, kind="ExternalInput").ap()
    xtb_d = nc.dram_tensor("xtb", [D, NS], bf16, kind="ExternalInput").ap()
    na_d = nc.dram_tensor("na_p", [D, R], f32, kind="ExternalInput").ap()
    nb_d = nc.dram_tensor("nb_p", [D, R], f32, kind="ExternalInput").ap()
    ng_d = nc.dram_tensor("ngrow", [1, R], f32, kind="ExternalInput").ap()
    one_d = nc.dram_tensor("ones1", [1, 128], f32, kind="ExternalInput").ap()
    eye_d = nc.dram_tensor("eye", [128, 128], bf16, kind="ExternalInput").ap()
    eye8_d = nc.dram_tensor("eye8", [128, 128], fp8, kind="ExternalInput").ap()
    eyr_d = nc.dram_tensor("eyerep", [128, 32 * 32], bf16, kind="ExternalInput").ap()
    c_d = nc.dram_tensor("cflat", [RT, 128, DJ], fp8, kind="ExternalInput").ap()
    cbo_d = nc.dram_tensor("cbo", [128, RT * (O + 4)], fp8, kind="ExternalInput").ap()
    out_d = nc.dram_tensor("out", [NS, O], f32, kind="ExternalOutput").ap()

    def r32(ap):
        return ap if ap.dtype == f32r else ap.bitcast(f32r)

    with tile.TileContext(nc) as tc:
        from contextlib import ExitStack

        with ExitStack() as ctx:
            konst = ctx.enter_context(tc.tile_pool(name="konst", bufs=1))
            cw = ctx.enter_context(tc.tile_pool(name="cw", bufs=1))
            stp = ctx.enter_context(tc.tile_pool(name="stp", bufs=1))
            work = ctx.enter_context(tc.tile_pool(name="work", bufs=3))
            small = ctx.enter_context(tc.tile_pool(name="small", bufs=4))
            psum = ctx.enter_context(tc.tile_pool(name="psum", bufs=2, space="PSUM"))

            # ---- input loads: strengths inputs first, then C pieces
            xt_sb = konst.tile([D, NS], f32r)
            xtb_sb = konst.tile([D, NS], bf16)
            na_sb = konst.tile([D, R], f32r)
            nb_sb = konst.tile([D, R], f32r)
            ng_sb = konst.tile([1, R], f32r)
            one_sb = konst.tile([1, 128], f32r)
            eye_sb = konst.tile([128, 128], bf16)
            eye8_sb = konst.tile([128, 128], fp8)
            eyr_sb = konst.tile([128, 32 * 32], bf16)
            cbo_sb = cw.tile([128, RT, O + 4], fp8)
            c_sb = cw.tile([128, RT, DJ], fp8)

            for h in range(2):
                hs = slice(h * 256, (h + 1) * 256)
                nc.sync.dma_start(xt_sb[:, hs], xt_d[:, hs].bitcast(f32r))
                nc.gpsimd.dma_start(xtb_sb[:, hs], xtb_d[:, hs])
            nc.sync.dma_start(na_sb[:], na_d.bitcast(f32r))
            nc.gpsimd.dma_start(nb_sb[:], nb_d.bitcast(f32r))
            nc.sync.dma_start(ng_sb[:], ng_d.bitcast(f32r))
            nc.sync.dma_start(one_sb[:], one_d.bitcast(f32r))
            nc.gpsimd.dma_start(eye_sb[:], eye_d)
            nc.gpsimd.dma_start(eye8_sb[:], eye8_d)
            nc.gpsimd.dma_start(eyr_sb[:], eyr_d)
            nc.sync.dma_start(cbo_sb[:], cbo_d.rearrange("p (rt o) -> p rt o", rt=RT))

            # C pieces ordered so unit groups (g, g+4) arrive first
            qi = 0
            for g in range(4):
                for pg in (g, g + 4):
                    for rt in range(RT):
                        base = pg * 1024
                        eng = nc.sync if qi % 2 == 0 else nc.gpsimd
                        qi += 1
                        eng.dma_start(
                            c_sb[:, rt, base : base + 1024],
                            c_d[rt, :, base : base + 1024],
                        )

            # ---- x^2 transposed (fp32r for the PE)
            x2t = konst.tile([D, NS], f32r)
            for hh in range(2):
                hs = slice(hh * 256, (hh + 1) * 256)
                nc.vector.tensor_tensor(
                    x2t[:, hs], xt_sb[:, hs].bitcast(f32),
                    xt_sb[:, hs].bitcast(f32), ALU.mult,
                )

            # ---- strengths: z[n,r] per n-tile, zmax, exp->fp8, transpose
            st3 = stp.tile([128, RT, NS], fp8)        # [r-in-kt, kt, n]
            nzmax = small.tile([128, NT], f32, name="nzmax")
            for nt in range(NT):
                z2 = psum.tile([128, 1024], f32, tag="big", name=f"z2_{nt}", bufs=2)
                nc.tensor.matmul(
                    z2[:, :R], r32(x2t[:, ts(nt, 128)]), r32(na_sb[:]),
                    start=True, stop=False,
                )
                nc.tensor.matmul(
                    z2[:, :R], r32(xt_sb[:, ts(nt, 128)]), r32(nb_sb[:]),
                    start=False, stop=False,
                )
                nc.tensor.matmul(
                    z2[:, :R], one_sb[:], ng_sb[:],
                    start=False, stop=True,
                )
                nc.vector.tensor_reduce(
                    nzmax[:, nt : nt + 1], z2[:, :R],
                    axis=mybir.AxisListType.X, op=ALU.max, negate=True,
                )
                st2 = small.tile([128, R], bf16, name=f"st2_{nt}", tag="st2", bufs=2)
                nc.scalar.activation(
                    st2[:], z2[:, :R], AF.Exp, bias=nzmax[:, nt : nt + 1], scale=1.0
                )
                stT = psum.tile([128, 256], bf16, tag="stT", name=f"stT{nt}", bufs=1)
                for kt in range(RT):
                    nc.tensor.transpose(
                        stT[:, ts(kt, 128)], st2[:, ts(kt, 128)],
                        eye_sb[:],
                    )
                nc.scalar.activation(
                    st3[:, :, ts(nt, 128)],
                    stT[:].rearrange("p (kt n) -> p kt n", kt=RT),
                    AF.Copy,
                )

            # eps64[n] = 64e-8 * exp(-zmax)  (ACT exp on the 4 columns)
            lneps = small.tile([128, 1], f32, name="lneps")
            nc.vector.memset(lneps[:], LN_EPS64)
            eps64 = small.tile([128, NT], f32, name="eps64")
            nc.scalar.activation(eps64[:], nzmax[:], AF.Exp, bias=lneps[:], scale=1.0)

            # ---- den + bias consequent per n-tile (one DoubleRow matmul)
            scalecs, tbs = [], []
            for nt in range(NT):
                dbp = psum.tile([128, 1024], f32, tag="big", name=f"dbp{nt}", bufs=2)
                nc.tensor.matmul(
                    dbp[:, : O + 4], st3[:, :, ts(nt, 128)], cbo_sb[:],
                    start=True, stop=True, perf_mode=DR,
                )
                denc = small.tile([128, 1], f32, name=f"denc{nt}")
                nc.vector.tensor_tensor(
                    denc[:], dbp[:, :1], eps64[:, nt : nt + 1], ALU.add
                )
                scalec = small.tile([128, 1], f32, name=f"scalec{nt}")
                nc.vector.reciprocal(scalec[:], denc[:])
                scalecs.append(scalec)
                tb_sb = small.tile([128, O], f32, name=f"tb{nt}", tag="tb")
                nc.scalar.activation(tb_sb[:], dbp[:, 4 : O + 4], AF.Copy)
                tbs.append(tb_sb)

            # ---- big contraction, transposed: per unit m -> T2 pair PSUM
            res = psum.tile([128, 1024], f32, tag="res", name="res", bufs=1)
            xbt = xtb_sb[:].unsqueeze(1).broadcast_to([128, 2, NS])
            for m in range(NUNIT):
                tp = psum.tile([128, 1024], f32, tag="big", name=f"tp{m}", bufs=2)
                for jp in range(2):
                    j = m + 32 * jp
                    nc.tensor.matmul(
                        tp[:, ts(jp, NS)],
                        c_sb[:, :, j * 128 : (j + 1) * 128],
                        st3[:],
                        start=True, stop=True, perf_mode=DR,
                    )
                prod = work.tile([128, 2, NS], bf16, name=f"prod{m}", tag="prod",
                                 bufs=4)
                route = ROUTES[m]
                tpv = tp[:].rearrange("p (jp n) -> p jp n", jp=2)
                if route == "b":
                    nc.vector.tensor_tensor(prod[:], tpv, xbt, ALU.mult)
                else:
                    tcp = work.tile([128, 2, NS], bf16, name=f"tcp{m}", tag="tcp",
                                    bufs=4)
                    nc.scalar.activation(tcp[:], tpv, AF.Copy)
                    eng = nc.gpsimd if route == "d" else nc.vector
                    eng.tensor_tensor(prod[:], tcp[:], xbt, ALU.mult)
                # d-reduction on the PE: ones-column matmuls accumulate row m
                for jp in range(2):
                    nc.tensor.matmul(
                        res[:32, ts(jp, NS)], eyr_sb[:, ts(m, 32)],
                        prod[:, jp, :],
                        start=(m == 0), stop=(m == NUNIT - 1),
                    )

            # ---- res -> logits[n, j] via PE transposes
            res_sb = stp.tile([32, 1024], bf16)
            nc.scalar.activation(res_sb[:], res[:32, :], AF.Copy)
            rv = res_sb[:].rearrange("p (jp n) -> p jp n", jp=2)
            logits = psum.tile([128, 4, O], bf16, tag="logits", name="logits", bufs=1)
            for nt in range(NT):
                for jp in range(2):
                    nc.tensor.transpose(
                        logits[:, nt, jp * 32 : (jp + 1) * 32],
                        rv[:, jp, ts(nt, 128)],
                        eye_sb[:32, :32],
                    )

            # ---- per n-tile: + bias, softmax, out
            for nt in range(NT):
                acc = small.tile([128, O], f32, name=f"acc{nt}")
                nc.vector.tensor_tensor(acc[:], logits[:, nt, :], tbs[nt][:], ALU.add)
                negm = small.tile([128, 1], f32, name=f"negm{nt}")
                nc.vector.tensor_reduce(
                    negm[:], acc[:], axis=mybir.AxisListType.X, op=ALU.max,
                    negate=True,
                )
                negmb = small.tile([128, 1], f32, name=f"negmb{nt}")
                nc.vector.tensor_tensor(negmb[:], negm[:], scalecs[nt][:], ALU.mult)
                exps = small.tile([128, O], f32, name=f"exps{nt}")
                sume = small.tile([128, 1], f32, name=f"sume{nt}")
                nc.scalar.activation(
                    exps[:], acc[:], AF.Exp, bias=negmb[:], scale=scalecs[nt][:],
                    accum_out=sume[:],
                )
                rs = small.tile([128, 1], f32, name=f"rs{nt}")
                nc.vector.reciprocal(rs[:], sume[:])
                osb = small.tile([128, O], f32, name=f"osb{nt}")
                nc.scalar.activation(osb[:], exps[:], AF.Copy, scale=rs[:])
                nc.sync.dma_start(out_d[ts(nt, 128), :], osb[:])

    nc.compile()
    return nc


def _prep_inputs(X, centers, sigmas, coeffs):
    """Host-side sharding + layout transforms (numpy only)."""
    X = np.ascontiguousarray(X, dtype=np.float32)
    centers = np.asarray(centers, dtype=np.float32)
    sigmas = np.asarray(sigmas, dtype=np.float32)
    coeffs = np.asarray(coeffs, dtype=np.float32)

    inv2s2 = 1.0 / (2.0 * sigmas * sigmas)            # [R, D]
    nA = np.ascontiguousarray(-inv2s2.T)              # [D, R]
    nB = np.ascontiguousarray((centers / (sigmas * sigmas)).T)  # [D, R]
    G = (centers * centers * inv2s2).sum(axis=1)      # [R]
    nG = np.ascontiguousarray(-G.reshape(1, R))
    ones1 = np.ones((1, 128), dtype=np.float32)
    eye = np.eye(128, dtype=np.float32).astype(BF16)
    eyerep = np.broadcast_to(
        np.eye(32, dtype=np.float32).reshape(1, 32, 32), (128, 32, 32)
    ).reshape(128, 32 * 32).astype(BF16)
    eyerep = np.ascontiguousarray(eyerep)

    # C in [r, (j, d)] layout, scaled by 64, fp8e4
    Cjd = np.ascontiguousarray(coeffs[:, :D, :].transpose(0, 2, 1))  # [R, O, D]
    Ck = np.ascontiguousarray(
        (CSCALE * Cjd).reshape(RT, 128, DJ).astype(FP8)
    )
    Cb = coeffs[:, D, :].reshape(RT, 128, O).transpose(1, 0, 2)  # [128, RT, O]
    Cbo = np.ones((128, RT, O + 4), dtype=np.float32)
    Cbo[:, :, 0] = CSCALE
    Cbo[:, :, 4:] = CSCALE * Cb
    Cbo = np.ascontiguousarray(Cbo.reshape(128, RT * (O + 4))).astype(FP8)

    in_maps = []
    for i in range(NCORES):
        Xs = X[i * NS : (i + 1) * NS]                  # [512, 128]
        xt = np.ascontiguousarray(Xs.T)                # [128, 512]
        in_maps.append(
            {
                "xt": xt,
                "xtb": xt.astype(BF16),
                "na_p": nA,
                "nb_p": nB,
                "ngrow": nG,
                "ones1": ones1,
                "eye": eye,
                "eye8": np.eye(128, dtype=np.float32).astype(FP8),
                "eyerep": eyerep,
                "cflat": Ck,
                "cbo": Cbo,
            }
        )
    return in_maps


def kernel(X, centers, sigmas, coeffs):
    from concourse.bass_utils import run_bass_kernel_spmd

    if "nc" not in _CACHE:
        _CACHE["nc"] = _build()
    nc = _CACHE["nc"]

    in_maps = _prep_inputs(X, centers, sigmas, coeffs)
    res = run_bass_kernel_spmd(nc, in_maps, list(range(NCORES)))
    out = np.concatenate([res.results[i]["out"] for i in range(NCORES)], axis=0)
    return out.astype(np.float32)


if __name__ == "__main__":
    rng = np.random.default_rng(0)
    X = rng.standard_normal((N, D), dtype=np.float32)
    centers = 0.5 * rng.standard_normal((R, D)).astype(np.float32)
    sigmas = (1.5 + rng.random((R, D))).astype(np.float32)
    coeffs = (0.02 * rng.standard_normal((R, D + 1, O))).astype(np.float32)
    out = kernel(X=X, centers=centers, sigmas=sigmas, coeffs=coeffs)
    print(out.shape, out.dtype, out.sum(axis=1)[:4])


# revision 18
# speedup vs baseline: 1.0669x; 1.0669x over previous
"""Trainium2 Bass kernel for nn_CustomANFIS (N=4096, D=128, R=256, O=64).

Math (reference):
  memb[n,r,d]  = exp(-(x[n,d]-c[r,d])^2 / (2 s[r,d]^2))
  str[n,r]     = prod_d memb = exp(z[n,r]) with
                 z[n,r] = -(sum_d x^2 A + sum_d x B + G),
                 A = 1/(2 s^2), B = -c/s^2, G = sum_d c^2/(2 s^2)
  den[n]       = sum_r str + 1e-8
  out          = softmax_j( (1/den) * sum_r str[n,r] * (X@C_r + b_r) )

Device algorithm (data-parallel over N across 8 cores, NS=512 rows/core):
  1. z[n,r] via 3 accumulating fp32r matmuls per n-tile ([n,256] PSUM,
     G folded in via a K=1 ones-row matmul); zmax = max_r z (DVE);
     st2[n,r] = exp(z - zmax) written as fp8e4 (ACT, per-partition bias).
     Per-n renorm keeps st in [0,1] for fp8; exactness restored via
     eps'[n] = 64e-8 * exp(-zmax) added to the 64-scaled den.
  2. st2 -> st3[r-in-ktile, kt, n] via PE transposes (fp8), ACT evac.
  3. den+bias consequent: one DoubleRow fp8 matmul per n-tile
     (lhsT=st3 slice, rhs=cbo with 64-scaled ones/bias cols).
  4. Big contraction TRANSPOSED (d on partitions): per j, one DoubleRow
     fp8 matmul T2_j[d, n] = sum_r C64[r,d,j] st[r,n], j-pairs (j, j+32)
     share a [128, 2, 512] PSUM tile.
  5. X-multiply: ACT evac pair -> bf16 (or DVE direct from PSUM), then
     DVE/GpSimd multiply by Xt[d,n] (broadcast over the pair slot).
  6. d-reduction ON THE PE: ones-column matmuls (lhsT = eye32 slice
     replicated over partitions) accumulate row m of res[32, 2, 512]
     = out^T. No DVE tree.
  7. res -> PE transposes -> logits[n, j] (bf16 PSUM); + bias (tb),
     softmax with scale folded (1/(64 den)), DMA out.
"""

import numpy as np
import ml_dtypes

N, D, R, O = 4096, 128, 256, 64
NCORES = 8
NS = N // NCORES          # 512 rows per core
NT = NS // 128            # 4 n-tiles per core
RT = R // 128             # 2 r k-tiles
DJ = D * O                # 8192
NUNIT = 32                # j-pair units: unit m covers j=m and j=m+32
CSCALE = 64.0
LN_EPS64 = float(np.log(CSCALE * 1e-8))   # ln(64e-8) = -14.2618...

# route per unit index 0..31: 'a' = ACT evac + DVE mult,
# 'b' = DVE mult direct from PSUM, 'c' = GPS mult direct from PSUM
ROUTES = ("a b d a b a d a " * 4).split()

_CACHE = {}
BF16 = ml_dtypes.bfloat16
FP8 = ml_dtypes.float8_e4m3


def _build():
    import concourse.bass as bass
    import concourse.tile as tile
    from concourse import bacc, mybir

    f32 = mybir.dt.float32
    f32r = mybir.dt.float32r
    bf16 = mybir.dt.bfloat16
    fp8 = mybir.dt.float8e4
    AF = mybir.ActivationFunctionType
    ALU = mybir.AluOpType
    DR = mybir.MatmulPerfMode.DoubleRow
    ts = bass.ts

    nc = bacc.Bacc(
        "TRN2", target_bir_lowering=False, debug=False, num_devices=NCORES
    )

    xt_d = nc.dram_tensor("xt", [D, NS], f32, kind="ExternalInput").ap()
    xtb_d = nc.dram_tensor("xtb", [D, NS], bf16, kind="ExternalInput").ap()
    na_d = nc.dram_tensor("na_p", [D, R], f32, kind="ExternalInput").ap()
    nb_d = nc.dram_tensor("nb_p", [D, R], f32, kind="ExternalInput").ap()
    ng_d = nc.dram_tensor("ngrow", [1, R], f32, kind="ExternalInput").ap()
    one_d = nc.dram_tensor("ones1", [1, 128], f32, kind="ExternalInput").ap()
    eye_d = nc.dram_tensor("eye", [128, 128], bf16, kind="ExternalInput").ap()
    eye8_d = nc.dram_tensor("eye8", [128, 128], fp8, kind="ExternalInput").ap()
    eyr_d = nc.dram_tensor("eyerep", [128, 32 * 32], bf16, kind="ExternalInput").ap()
    c_d = nc.dram_tensor("cflat", [RT, 128, DJ], fp8, kind="ExternalInput").ap()
    cbo_d = nc.dram_tensor("cbo", [128, RT * (O + 4)], fp8, kind="ExternalInput").ap()
    out_d = nc.dram_tensor("out", [NS, O], f32, kind="ExternalOutput").ap()

    def r32(ap):
        return ap if ap.dtype == f32r else ap.bitcast(f32r)

    with tile.TileContext(nc) as tc:
        from contextlib import ExitStack

        with ExitStack() as ctx:
            konst = ctx.enter_context(tc.tile_pool(name="konst", bufs=1))
            cw = ctx.enter_context(tc.tile_pool(name="cw", bufs=1))
            stp = ctx.enter_context(tc.tile_pool(name="stp", bufs=1))
            work = ctx.enter_context(tc.tile_pool(name="work", bufs=3))
            small = ctx.enter_context(tc.tile_pool(name="small", bufs=4))
            psum = ctx.enter_context(tc.tile_pool(name="psum", bufs=2, space="PSUM"))

            # ---- input loads: strengths inputs first, then C pieces
            xt_sb = konst.tile([D, NS], f32r)
            xtb_sb = konst.tile([D, NS], bf16)
            na_sb = konst.tile([D, R], f32r)
            nb_sb = konst.tile([D, R], f32r)
            ng_sb = konst.tile([1, R], f32r)
            one_sb = konst.tile([1, 128], f32r)
            eye_sb = konst.tile([128, 128], bf16)
            eye8_sb = konst.tile([128, 128], fp8)
            eyr_sb = konst.tile([128, 32 * 32], bf16)
            cbo_sb = cw.tile([128, RT, O + 4], fp8)
            c_sb = cw.tile([128, RT, DJ], fp8)

            for h in range(2):
                hs = slice(h * 256, (h + 1) * 256)
                nc.sync.dma_start(xt_sb[:, hs], xt_d[:, hs].bitcast(f32r))
                nc.gpsimd.dma_start(xtb_sb[:, hs], xtb_d[:, hs])
            nc.sync.dma_start(na_sb[:], na_d.bitcast(f32r))
            nc.gpsimd.dma_start(nb_sb[:], nb_d.bitcast(f32r))
            nc.sync.dma_start(ng_sb[:], ng_d.bitcast(f32r))
            nc.sync.dma_start(one_sb[:], one_d.bitcast(f32r))
            nc.gpsimd.dma_start(eye_sb[:], eye_d)
            nc.gpsimd.dma_start(eye8_sb[:], eye8_d)
            nc.gpsimd.dma_start(eyr_sb[:], eyr_d)
            nc.sync.dma_start(cbo_sb[:], cbo_d.rearrange("p (rt o) -> p rt o", rt=RT))

            # C pieces ordered so unit groups (g, g+4) arrive first
            qi = 0
            for g in range(4):
                for pg in (g, g + 4):
                    for rt in range(RT):
                        base = pg * 1024
                        eng = nc.sync if qi % 2 == 0 else nc.gpsimd
                        qi += 1
                        eng.dma_start(
                            c_sb[:, rt, base : base + 1024],
                            c_d[rt, :, base : base + 1024],
                        )

            # ---- x^2 transposed (fp32r for the PE)
            x2t = konst.tile([D, NS], f32r)
            for hh in range(2):
                hs = slice(hh * 256, (hh + 1) * 256)
                nc.vector.tensor_tensor(
                    x2t[:, hs], xt_sb[:, hs].bitcast(f32),
                    xt_sb[:, hs].bitcast(f32), ALU.mult,
                )

            # ---- strengths: z[n,r] per n-tile, zmax, exp->fp8, transpose
            st3 = stp.tile([128, RT, NS], fp8)        # [r-in-kt, kt, n]
            nzmax = small.tile([128, NT], f32, name="nzmax")
            for nt in range(NT):
                z2 = psum.tile([128, 1024], f32, tag="big", name=f"z2_{nt}", bufs=2)
                nc.tensor.matmul(
                    z2[:, :R], r32(x2t[:, ts(nt, 128)]), r32(na_sb[:]),
                    start=True, stop=False,
                )
                nc.tensor.matmul(
                    z2[:, :R], r32(xt_sb[:, ts(nt, 128)]), r32(nb_sb[:]),
                    start=False, stop=False,
                )
                nc.tensor.matmul(
                    z2[:, :R], one_sb[:], ng_sb[:],
                    start=False, stop=True,
                )
                nc.vector.tensor_reduce(
                    nzmax[:, nt : nt + 1], z2[:, :R],
                    axis=mybir.AxisListType.X, op=ALU.max, negate=True,
                )
                st2 = small.tile([128, R], bf16, name=f"st2_{nt}", tag="st2", bufs=2)
                nc.scalar.activation(
                    st2[:], z2[:, :R], AF.Exp, bias=nzmax[:, nt : nt + 1], scale=1.0
                )
                stT = psum.tile([128, 256], bf16, tag="stT", name=f"stT{nt}", bufs=1)
                for kt in range(RT):
                    nc.tensor.transpose(
                        stT[:, ts(kt, 128)], st2[:, ts(kt, 128)],
                        eye_sb[:],
                    )
                nc.scalar.activation(
                    st3[:, :, ts(nt, 128)],
                    stT[:].rearrange("p (kt n) -> p kt n", kt=RT),
                    AF.Copy,
                )

            # eps64[n] = 64e-8 * exp(-zmax)  (ACT exp on the 4 columns)
            lneps = small.tile([128, 1], f32, name="lneps")
            nc.vector.memset(lneps[:], LN_EPS64)
            eps64 = small.tile([128, NT], f32, name="eps64")
            nc.scalar.activation(eps64[:], nzmax[:], AF.Exp, bias=lneps[:], scale=1.0)

            # ---- den + bias consequent per n-tile (one DoubleRow matmul)
            scalecs, tbs = [], []
            for nt in range(NT):
                dbp = psum.tile([128, 1024], f32, tag="big", name=f"dbp{nt}", bufs=2)
                nc.tensor.matmul(
                    dbp[:, : O + 4], st3[:, :, ts(nt, 128)], cbo_sb[:],
                    start=True, stop=True, perf_mode=DR,
                )
                denc = small.tile([128, 1], f32, name=f"denc{nt}")
                nc.vector.tensor_tensor(
                    denc[:], dbp[:, :1], eps64[:, nt : nt + 1], ALU.add
                )
                scalec = small.tile([128, 1], f32, name=f"scalec{nt}")
                nc.vector.reciprocal(scalec[:], denc[:])
                scalecs.append(scalec)
                tb_sb = small.tile([128, O], f32, name=f"tb{nt}", tag="tb")
                nc.scalar.activation(tb_sb[:], dbp[:, 4 : O + 4], AF.Copy)
                tbs.append(tb_sb)

            # ---- big contraction, transposed: per unit m -> T2 pair PSUM
            res = psum.tile([128, 1024], f32, tag="res", name="res", bufs=1)
            xbt = xtb_sb[:].unsqueeze(1).broadcast_to([128, 2, NS])
            for m in range(NUNIT):
                tp = psum.tile([128, 1024], f32, tag="big", name=f"tp{m}", bufs=2)
                for jp in range(2):
                    j = m + 32 * jp
                    nc.tensor.matmul(
                        tp[:, ts(jp, NS)],
                        c_sb[:, :, j * 128 : (j + 1) * 128],
                        st3[:],
                        start=True, stop=True, perf_mode=DR,
                    )
                prod = work.tile([128, 2, NS], bf16, name=f"prod{m}", tag="prod",
                                 bufs=4)
                route = ROUTES[m]
                tpv = tp[:].rearrange("p (jp n) -> p jp n", jp=2)
                if route == "b":
                    nc.vector.tensor_tensor(prod[:], tpv, xbt, ALU.mult)
                else:
                    tcp = work.tile([128, 2, NS], bf16, name=f"tcp{m}", tag="tcp",
                                    bufs=4)
                    nc.scalar.activation(tcp[:], tpv, AF.Copy)
                    eng = nc.gpsimd if route == "d" else nc.vector
                    eng.tensor_tensor(prod[:], tcp[:], xbt, ALU.mult)
                # d-reduction on the PE: ones-column matmuls accumulate row m
                for jp in range(2):
                    nc.tensor.matmul(
                        res[:32, ts(jp, NS)], eyr_sb[:, ts(m, 32)],
                        prod[:, jp, :],
                        start=(m == 0), stop=(m == NUNIT - 1),
                    )

            # ---- res -> logits[n, j] via PE transposes
            res_sb = stp.tile([32, 1024], bf16)
            nc.scalar.activation(res_sb[:], res[:32, :], AF.Copy)
            rv = res_sb[:].rearrange("p (jp n) -> p jp n", jp=2)
            logits = psum.tile([128, 4, O], bf16, tag="logits", name="logits", bufs=1)
            for nt in range(NT):
                for jp in range(2):
                    nc.tensor.transpose(
                        logits[:, nt, jp * 32 : (jp + 1) * 32],
                        rv[:, jp, ts(nt, 128)],
                        eye_sb[:32, :32],
                    )

            # ---- per n-tile: + bias, softmax, out
            for nt in range(NT):
                acc = small.tile([128, O], f32, name=f"acc{nt}")
                nc.vector.tensor_tensor(acc[:], logits[:, nt, :], tbs[nt][:], ALU.add)
                negm = small.tile([128, 1], f32, name=f"negm{nt}")
                nc.vector.tensor_reduce(
                    negm[:], acc[:], axis=mybir.AxisListType.X, op=ALU.max,
                    negate=True,
                )
                negmb = small.tile([128, 1], f32, name=f"negmb{nt}")
                nc.vector.tensor_tensor(negmb[:], negm[:], scalecs[nt][:], ALU.mult)
                exps = small.tile([128, O], f32, name=f"exps{nt}")
                sume = small.tile([128, 1], f32, name=f"sume{nt}")
                nc.scalar.activation(
                    exps[:], acc[:], AF.Exp, bias=negmb[:], scale=scalecs[nt][:],
                    accum_out=sume[:],
                )
                rs = small.tile([128, 1], f32, name=f"rs{nt}")
                nc.vector.reciprocal(rs[:], sume[:])
                osb = small.tile([128, O], f32, name=f"osb{nt}")
                nc.scalar.activation(osb[:], exps[:], AF.Copy, scale=rs[:])
                nc.sync.dma_start(out_d[ts(nt, 128), :], osb[:])

    nc.compile()
    return nc


def _prep_inputs(X, centers, sigmas, coeffs):
    """Host-side sharding + layout transforms (numpy only)."""
    X = np.ascontiguousarray(X, dtype=np.float32)
    centers = np.asarray(centers, dtype=np.float32)
    sigmas = np.asarray(sigmas, dtype=np.float32)
    coeffs = np.asarray(coeffs, dtype=np.float32)

    inv2s2 = 1.0 / (2.0 * sigmas * sigmas)            # [R, D]
    nA = np.ascontiguousarray(-inv2s2.T)              # [D, R]
    nB = np.ascontiguousarray((centers / (sigmas * sigmas)).T)  # [D, R]
    G = (centers * centers * inv2s2).sum(axis=1)      # [R]
    nG = np.ascontiguousarray(-G.reshape(1, R))
    ones1 = np.ones((1, 128), dtype=np.float32)
    eye = np.eye(128, dtype=np.float32).astype(BF16)
    eyerep = np.broadcast_to(
        np.eye(32, dtype=np.float32).reshape(1, 32, 32), (128, 32, 32)
    ).reshape(128, 32 * 32).astype(BF16)
    eyerep = np.ascontiguousarray(eyerep)

    # C in [r, (j, d)] layout, scaled by 64, fp8e4
    Cjd = np.ascontiguousarray(coeffs[:, :D, :].transpose(0, 2, 1))  # [R, O, D]
    Ck = np.ascontiguousarray(
        (CSCALE * Cjd).reshape(RT, 128, DJ).astype(FP8)
    )
    Cb = coeffs[:, D, :].reshape(RT, 128, O).transpose(1, 0, 2)  # [128, RT, O]
    Cbo = np.ones((128, RT, O + 4), dtype=np.float32)
    Cbo[:, :, 0] = CSCALE
    Cbo[:, :, 4:] = CSCALE * Cb
    Cbo = np.ascontiguousarray(Cbo.reshape(128, RT * (O + 4))).astype(FP8)

    in_maps = []
    for i in range(NCORES):
        Xs = X[i * NS : (i + 1) * NS]                  # [512, 128]
        xt = np.ascontiguousarray(Xs.T)                # [128, 512]
        in_maps.append(
            {
                "xt": xt,
                "xtb": xt.astype(BF16),
                "na_p": nA,
                "nb_p": nB,
                "ngrow": nG,
                "ones1": ones1,
                "eye": eye,
                "eye8": np.eye(128, dtype=np.float32).astype(FP8),
                "eyerep": eyerep,
                "cflat": Ck,
                "cbo": Cbo,
            }
        )
    return in_maps


def kernel(X, centers, sigmas, coeffs):
    from concourse.bass_utils import run_bass_kernel_spmd

    if "nc" not in _CACHE:
        _CACHE["nc"] = _build()
    nc = _CACHE["nc"]

    in_maps = _prep_inputs(X, centers, sigmas, coeffs)
    res = run_bass_kernel_spmd(nc, in_maps, list(range(NCORES)))
    out = np.concatenate([res.results[i]["out"] for i in range(NCORES)], axis=0)
    return out.astype(np.float32)


if __name__ == "__main__":
    rng = np.random.default_rng(0)
    X = rng.standard_normal((N, D), dtype=np.float32)
    centers = 0.5 * rng.standard_normal((R, D)).astype(np.float32)
    sigmas = (1.5 + rng.random((R, D))).astype(np.float32)
    coeffs = (0.02 * rng.standard_normal((R, D + 1, O))).astype(np.float32)
    out = kernel(X=X, centers=centers, sigmas=sigmas, coeffs=coeffs)
    print(out.shape, out.dtype, out.sum(axis=1)[:4])


# revision 19
# speedup vs baseline: 1.0795x; 1.0118x over previous
"""Trainium2 Bass kernel for nn_CustomANFIS (N=4096, D=128, R=256, O=64).

Math (reference):
  memb[n,r,d]  = exp(-(x[n,d]-c[r,d])^2 / (2 s[r,d]^2))
  str[n,r]     = prod_d memb = exp(-q[n,r]) with
                 q[n,r] = sum_d x^2[n,d]*A[d,r] + sum_d x[n,d]*B[d,r] + G[r],
                 A = 1/(2 s^2), B = -c/s^2, G = sum_d c^2/(2 s^2)
  den[n]       = sum_r str + 1e-8
  W[n,r,:]     = x[n,:] @ coeffs[r,:D,:] + coeffs[r,D,:]
  out          = softmax_j( (1/den) * sum_r str[n,r] * W[n,r,j] )

Device algorithm (data-parallel over N across 8 cores):
  1. strengths^T [r (2 part-tiles), n=512] via 2 fp32r accumulating matmuls
     + ACT exp (per-partition bias=-G), written as bf16.
  2. den column per n-tile via matmul(lhsT = sT-slice, rhs = ones).
  3. T[n, (j,d)] = sum_r sT[r,n] * C[r, (j,d)] in bf16 (16 chunks of 512 =
     4 j x 128 d per n-tile, PSUM-accumulated over the 2 r K-tiles), plus
     Tb[n,j] = sum_r sT[r,n]*Cb[r,j].
  4. prod[n, j, d] = X[n,d] * T[n,j,d]: ACT casts PSUM->SBUF bf16, then
     DVE (or GPSIMD for GPS_SET chunks) multiplies; DVE reads PSUM fp32
     directly for FP32_DIRECT chunks; then a bf16 tree-reduction over d
     and a fused (tree + Tb) add -> acc[n,j].
  5. logits = acc/den; softmax over j via ACT exp + accum_out.

Strengths-input DMAs (xt/a/b/ng/xn) are issued BEFORE the first C chunk so
the strengths matmuls start ~8us earlier and the chunk pipeline ramps
while C streams in.
"""

import numpy as np
import ml_dtypes

N, D, R, O = 4096, 128, 256, 64
NCORES = 8
NS = N // NCORES          # 512 rows per core
NT = NS // 128            # 4 n-tiles per core
RT = R // 128             # 2 r k-tiles
DJ = D * O                # 8192
CHUNK = 1024              # 2 PSUM banks per chunk
NCHUNK = DJ // CHUNK      # 8 chunks (8 j x 128 d each)
JPC = CHUNK // D          # 8 j per chunk
MM = 512                  # moving free dim per matmul

# per n-tile chunk split: the first FP32_DIRECT chunks are multiplied by DVE
# straight from PSUM (fp32, 1x); the rest are ACT-cast to bf16 SBUF and
# multiplied by DVE at 2x, except chunks in GPS_SET (global index nt*8+c)
# whose multiply runs on GPSIMD to offload the vector engine.
FP32_DIRECT = 1
GPS_SET = frozenset({5, 11, 14, 20, 26, 30})

_CACHE = {}
BF16 = ml_dtypes.bfloat16


def _build():
    import concourse.bass as bass
    import concourse.tile as tile
    from concourse import bacc, mybir

    f32 = mybir.dt.float32
    f32r = mybir.dt.float32r
    bf16 = mybir.dt.bfloat16
    AF = mybir.ActivationFunctionType
    ALU = mybir.AluOpType
    ts = bass.ts

    nc = bacc.Bacc(
        "TRN2", target_bir_lowering=False, debug=False, num_devices=NCORES
    )

    xt_d = nc.dram_tensor("xt", [D, NS], f32, kind="ExternalInput").ap()
    xn_d = nc.dram_tensor("xn", [128, NT * D], bf16, kind="ExternalInput").ap()
    a_d = nc.dram_tensor("a_p", [D, R], f32, kind="ExternalInput").ap()
    b_d = nc.dram_tensor("b_p", [D, R], f32, kind="ExternalInput").ap()
    ng_d = nc.dram_tensor("negg", [128, RT], f32, kind="ExternalInput").ap()
    c_d = nc.dram_tensor("cflat", [RT, 128, DJ], bf16, kind="ExternalInput").ap()
    cbo_d = nc.dram_tensor("cbo", [128, RT * (O + 2)], bf16, kind="ExternalInput").ap()
    out_d = nc.dram_tensor("out", [NS, O], f32, kind="ExternalOutput").ap()

    def r32(ap):
        return ap if ap.dtype == f32r else ap.bitcast(f32r)

    with tile.TileContext(nc) as tc:
        from contextlib import ExitStack

        with ExitStack() as ctx:
            konst = ctx.enter_context(tc.tile_pool(name="konst", bufs=1))
            cw = ctx.enter_context(tc.tile_pool(name="cw", bufs=1))
            stp = ctx.enter_context(tc.tile_pool(name="stp", bufs=1))
            prodp = ctx.enter_context(tc.tile_pool(name="prodp", bufs=3))
            small = ctx.enter_context(tc.tile_pool(name="small", bufs=4))
            psum = ctx.enter_context(tc.tile_pool(name="psum", bufs=2, space="PSUM"))

            # ---- parameter / input loads (strengths inputs first)
            xt_sb = konst.tile([D, NS], f32r)
            for q in range(4):
                eng = nc.sync if q % 2 == 0 else nc.gpsimd
                eng.dma_start(
                    xt_sb[:, q * 128 : (q + 1) * 128],
                    xt_d[:, q * 128 : (q + 1) * 128].bitcast(f32r),
                )
            a_sb = konst.tile([D, R], f32r)
            b_sb = konst.tile([D, R], f32r)
            ng_sb = konst.tile([128, RT], f32)

            # ---- big weights
            c_sb = cw.tile([128, RT * DJ], bf16)
            cbo_sb = cw.tile([128, RT * (O + 2)], bf16)
            xn_sb = konst.tile([128, NT * D], bf16)

            nc.sync.dma_start(b_sb[:, 0:128], b_d[:, 0:128].bitcast(f32r))
            nc.gpsimd.dma_start(b_sb[:, 128:256], b_d[:, 128:256].bitcast(f32r))
            nc.sync.dma_start(a_sb[:, 0:128], a_d[:, 0:128].bitcast(f32r))
            nc.gpsimd.dma_start(a_sb[:, 128:256], a_d[:, 128:256].bitcast(f32r))
            nc.sync.dma_start(ng_sb[:], ng_d)
            nc.gpsimd.dma_start(xn_sb[:], xn_d)

            def c_dma(c, qi):
                for half in range(CHUNK // MM):
                    for rt in range(RT):
                        base = c * CHUNK + half * MM
                        eng = nc.sync if qi % 2 == 0 else nc.gpsimd
                        qi += 1
                        eng.dma_start(
                            c_sb[:, rt * DJ + base : rt * DJ + base + MM],
                            c_d[rt, :, base : base + MM],
                        )
                return qi

            qi = c_dma(0, 0)
            nc.sync.dma_start(cbo_sb[:], cbo_d)
            for c in range(1, NCHUNK):
                qi = c_dma(c, qi)

            # x^2 transposed (written rounded-to-fp32r for the PE)
            x2t = konst.tile([D, NS], f32r)
            for hh in range(2):
                hs = slice(hh * 256, (hh + 1) * 256)
                nc.vector.tensor_tensor(
                    x2t[:, hs], xt_sb[:, hs].bitcast(f32),
                    xt_sb[:, hs].bitcast(f32), ALU.mult,
                )

            # ---- strengths^T: [r-tile partitions, n free], bf16
            st_tiles = []
            for rt in range(RT):
                sps = psum.tile([128, CHUNK], f32, tag="bank2", name=f"sps{rt}", bufs=4)
                nc.tensor.matmul(
                    sps[:, :NS], r32(b_sb[:, ts(rt, 128)]), r32(xt_sb[:]),
                    start=True, stop=False,
                )
                nc.tensor.matmul(
                    sps[:, :NS], r32(a_sb[:, ts(rt, 128)]), r32(x2t[:]),
                    start=False, stop=True,
                )
                st = stp.tile([128, NS], bf16, name=f"st{rt}")
                nc.scalar.activation(
                    st[:], sps[:, :NS], AF.Exp, bias=ng_sb[:, rt : rt + 1], scale=-1.0
                )
                st_tiles.append(st)

            # ---- den + bias consequent for all n-tiles (merged rhs)
            scalecs, tbs = [], []
            for nt in range(NT):
                st_n = [st[:, ts(nt, 128)] for st in st_tiles]
                dbp = psum.tile([128, CHUNK], f32, tag="bank2", name=f"dbp{nt}", bufs=4)
                nc.tensor.matmul(
                    dbp[:, : O + 2], st_n[0], cbo_sb[:, 0 : O + 2],
                    start=True, stop=False,
                )
                nc.tensor.matmul(
                    dbp[:, : O + 2], st_n[1], cbo_sb[:, O + 2 : 2 * (O + 2)],
                    start=False, stop=True,
                )
                denc = small.tile([128, 1], f32, name=f"denc{nt}")
                nc.vector.tensor_scalar_add(denc[:], dbp[:, :1], 1e-8)
                scalec = small.tile([128, 1], f32, name=f"scalec{nt}")
                nc.vector.reciprocal(scalec[:], denc[:])
                scalecs.append(scalec)
                tb_sb = small.tile([128, O], f32, name=f"tb{nt}", tag="tb")
                nc.scalar.activation(tb_sb[:], dbp[:, 2 : O + 2], AF.Copy)
                tbs.append(tb_sb)

            # ---- per n-tile pipeline
            for nt in range(NT):
                st_n = [st[:, ts(nt, 128)] for st in st_tiles]
                scalec = scalecs[nt]
                tb_sb = tbs[nt]

                # prod layout: [n, j, d] (d contiguous)
                prod = prodp.tile([128, O, D], bf16, name=f"prod{nt}", tag="prod")

                xrow = xn_sb[:, ts(nt, D)]  # [128 n, 128 d] bf16
                xb = xrow.unsqueeze(1).broadcast_to([128, JPC, D])
                sbuf_s = small.tile([128, O, D // 2], bf16, tag="tree", name=f"s{nt}")

                def emit_tree(jlo, jhi):
                    sg = sbuf_s[:, jlo:jhi, :]
                    nc.vector.tensor_tensor(
                        sg[:, :, :], prod[:, jlo:jhi, 0 : D // 2],
                        prod[:, jlo:jhi, D // 2 : D], ALU.add,
                    )
                    h = D // 2
                    while h > 1:
                        h //= 2
                        nc.vector.tensor_tensor(
                            sg[:, :, 0:h], sg[:, :, 0:h], sg[:, :, h : 2 * h],
                            ALU.add,
                        )

                for c in range(NCHUNK):
                    tps = psum.tile(
                        [128, CHUNK], f32, tag="bank2", name=f"tps{nt}_{c}", bufs=4
                    )
                    for rt in range(RT):
                        for half in range(CHUNK // MM):
                            hsl = slice(half * MM, (half + 1) * MM)
                            base = c * CHUNK + half * MM
                            nc.tensor.matmul(
                                tps[:, hsl], st_n[rt],
                                c_sb[:, rt * DJ + base : rt * DJ + base + MM],
                                start=(rt == 0), stop=(rt == RT - 1),
                            )
                    tview = tps[:].rearrange("p (j d) -> p j d", j=JPC)
                    oview = prod[:, c * JPC : (c + 1) * JPC, :]  # [128, 8, 128]
                    if c < FP32_DIRECT:
                        nc.vector.tensor_tensor(oview, tview, xb, ALU.mult)
                    else:
                        tcp = small.tile(
                            [128, JPC, D], bf16, tag="tcp", name=f"tcp{nt}_{c}",
                            bufs=6,
                        )
                        nc.scalar.activation(tcp[:], tps[:], AF.Copy)
                        eng = (
                            nc.gpsimd if (nt * NCHUNK + c) in GPS_SET else nc.vector
                        )
                        eng.tensor_tensor(oview, tcp[:], xb, ALU.mult)

                    if nt == NT - 1 and c == NCHUNK // 2 - 1:
                        emit_tree(0, O // 2)

                # tree-reduction over d (bf16, contiguous innermost)
                if nt == NT - 1:
                    emit_tree(O // 2, O)
                else:
                    emit_tree(0, O)

                # acc = tree + Tb  (fused, reads Tb straight from PSUM)
                acc = small.tile([128, O], f32, name=f"acc{nt}")
                nc.vector.scalar_tensor_tensor(
                    acc[:], sbuf_s[:, :, 0], 1.0, tb_sb[:], ALU.mult, ALU.add
                )

                # softmax over j of logits = acc/den, fused:
                # exp(acc*scalec - max(acc)*scalec), max taken on unscaled acc
                negm = small.tile([128, 1], f32, name=f"negm{nt}")
                nc.vector.tensor_reduce(
                    negm[:], acc[:], axis=mybir.AxisListType.X, op=ALU.max,
                    negate=True,
                )
                negmb = small.tile([128, 1], f32, name=f"negmb{nt}")
                nc.vector.tensor_tensor(negmb[:], negm[:], scalec[:], ALU.mult)
                exps = small.tile([128, O], f32, name=f"exps{nt}")
                sume = small.tile([128, 1], f32, name=f"sume{nt}")
                nc.scalar.activation(
                    exps[:], acc[:], AF.Exp, bias=negmb[:], scale=scalec[:],
                    accum_out=sume[:],
                )
                rs = small.tile([128, 1], f32, name=f"rs{nt}")
                nc.vector.reciprocal(rs[:], sume[:])
                osb = small.tile([128, O], f32, name=f"osb{nt}")
                nc.scalar.activation(osb[:], exps[:], AF.Copy, scale=rs[:])
                nc.sync.dma_start(out_d[ts(nt, 128), :], osb[:])

    nc.compile()
    return nc


def _prep_inputs(X, centers, sigmas, coeffs):
    """Host-side sharding + layout transforms (numpy only)."""
    X = np.ascontiguousarray(X, dtype=np.float32)
    centers = np.asarray(centers, dtype=np.float32)
    sigmas = np.asarray(sigmas, dtype=np.float32)
    coeffs = np.asarray(coeffs, dtype=np.float32)

    inv2s2 = 1.0 / (2.0 * sigmas * sigmas)            # [R, D]
    A = np.ascontiguousarray(inv2s2.T)                # [D, R]
    B = np.ascontiguousarray((-centers / (sigmas * sigmas)).T)  # [D, R]
    G = (centers * centers * inv2s2).sum(axis=1)      # [R]
    negG = np.ascontiguousarray(-G.reshape(RT, 128).T)  # [128, RT]

    # C in [r, (j, d)] layout, bf16
    Cjd = np.ascontiguousarray(coeffs[:, :D, :].transpose(0, 2, 1))  # [R, O, D]
    Ck = np.ascontiguousarray(Cjd.reshape(RT, 128, DJ).astype(BF16))
    Cb = coeffs[:, D, :].reshape(RT, 128, O).transpose(1, 0, 2)  # [128, RT, O]
    Cbo = np.ones((128, RT, O + 2), dtype=np.float32)
    Cbo[:, :, 2:] = Cb
    Cbo = np.ascontiguousarray(Cbo.reshape(128, RT * (O + 2))).astype(BF16)

    in_maps = []
    for i in range(NCORES):
        Xs = X[i * NS : (i + 1) * NS]                  # [512, 128]
        xt = np.ascontiguousarray(Xs.T)                # [128, 512]
        xn = np.ascontiguousarray(
            Xs.reshape(NT, 128, D).transpose(1, 0, 2).reshape(128, NT * D)
        ).astype(BF16)
        in_maps.append(
            {
                "xt": xt,
                "xn": xn,
                "a_p": A,
                "b_p": B,
                "negg": negG,
                "cflat": Ck,
                "cbo": Cbo,
            }
        )
    return in_maps


def kernel(X, centers, sigmas, coeffs):
    from concourse.bass_utils import run_bass_kernel_spmd

    if "nc" not in _CACHE:
        _CACHE["nc"] = _build()
    nc = _CACHE["nc"]

    in_maps = _prep_inputs(X, centers, sigmas, coeffs)
    res = run_bass_kernel_spmd(nc, in_maps, list(range(NCORES)))
    out = np.concatenate([res.results[i]["out"] for i in range(NCORES)], axis=0)
    return out.astype(np.float32)


if __name__ == "__main__":
    rng = np.random.default_rng(0)
    X = rng.standard_normal((N, D), dtype=np.float32)
    centers = 0.5 * rng.standard_normal((R, D)).astype(np.float32)
    sigmas = (1.5 + rng.random((R, D))).astype(np.float32)
    coeffs = (0.02 * rng.standard_normal((R, D + 1, O))).astype(np.float32)
    out = kernel(X=X, centers=centers, sigmas=sigmas, coeffs=coeffs)
    print(out.shape, out.dtype, out.sum(axis=1)[:4])
